# revision 42
# baseline (speedup 1.0000x reference)
"""Trainium2 Bass kernel for GaussianKernelConv.

Math: out[b,n,p] = mean_k exp(-||x[b,n,k,:] - kp[p,:]||^2 / (2 sigma^2))

Device strategy (per core, one batch b of N=8192 neighborhoods):
  exp argument is affine in (x, ||x||^2):
      arg = sum_c (kp[p,c]/s^2) * x_c  +  (-1/2s^2) * ||x||^2  +  bias_p
      bias_p = -||kp_p||^2/(2 s^2) - ln(K)       (folds the 1/K of the mean)
  - Host packs [x0, x1, x2, ||x||^2] per neighbor into the 4 contraction
    rows of a slot, so ONE matmul per 32-row group computes the full arg
    (the baseline needed a second accumulation pass for the x^2 term).
  - TensorE: per pass (512 n), 4 matmuls (one per 32-row group) into one
    4-bank PSUM tile. fp16 in, fp32 PSUM.
  - ScalarE: exp over the 4 banks in one activation (per-partition bias),
    fp16 out. This is the throughput wall (~1.9us per 2048 cols).
  - VectorE: fp16 (2x mode) binary-tree adds reduce K=32 -> 1.
  - DMA: one leading transfer carries [weights | bias | pass-0 data] so a
    single completion unblocks the first matmul; the rest of the input
    prefetches in 7 chunks (all issued up front, one SBUF tile each);
    fp16 output in chunks that shrink toward the end of the pipeline.

Sharding: data-parallel over batch B=8 -> 8 cores, one batch each.
"""

import sys

for _p in ("/opt/trn_rl_repo",):
    if _p not in sys.path:
        sys.path.insert(0, _p)

import numpy as np

B, N, K, C, P = 8, 8192, 32, 3, 16
NPASS = 16          # n-passes per batch; each pass covers 512 n
NGRP = 4            # matmul groups (PSUM bank quarters) per pass
H = 16              # n-high per column block
NSLOT = 8           # n-slots per column (block-diagonal points)
SLOTS = 4           # rows per point: x0, x1, x2, ||x||^2
COLS = K * H        # 512 columns per matmul

# Input DMA chunk boundaries (in passes). Pass 0 rides the leading const
# DMA (with wx and bias) so a single transfer unblocks the first matmul.
# First chunks are small so the pipeline fills fast; every chunk gets its
# own SBUF tile (total input is only ~16KB/partition) so all loads issue
# immediately with no WAR stalls.
IN_CHUNKS = [(1, 2), (2, 3), (3, 4), (4, 6), (6, 8), (8, 12), (12, 16)]
# Output DMA chunk boundaries: big early, small late so the drain after the
# last pass only carries 64 columns.
OUT_CHUNKS = [(0, 4), (4, 8), (8, 12), (12, 14), (14, 15), (15, 16)]
CONST_COLS = 130    # 128 wx cols + 2 cols carrying the f32 bias bitcast

_CACHE = {}


def _build_nc():
    from concourse import bacc, mybir
    from concourse.tile import TileContext

    f16, f32 = mybir.dt.float16, mybir.dt.float32
    Alu = mybir.AluOpType
    Act = mybir.ActivationFunctionType

    nc = bacc.Bacc(None, target_bir_lowering=False)
    # cx: [wx | bias(f32 as 2xf16) | pass-0 data] in one leading transfer.
    cx = nc.declare_dram_parameter("cx", [128, CONST_COLS + COLS], f16, isOutput=False)
    xin = nc.declare_dram_parameter("xin", [128, NPASS * COLS], f16, isOutput=False)
    out = nc.declare_dram_parameter("out", [128, NPASS * 64], f16, isOutput=True)

    with TileContext(nc) as tc:
        with (
            tc.tile_pool(name="const", bufs=1) as cpool,
            tc.tile_pool(name="xp", bufs=len(IN_CHUNKS)) as xpool,
            tc.tile_pool(name="ep", bufs=3) as epool,
            tc.tile_pool(name="tp", bufs=3) as tpool,
            tc.tile_pool(name="op", bufs=3) as opool,
            tc.tile_pool(name="ps", bufs=2, space="PSUM") as ppool,
        ):
            # Depless warmup activation: forces the Exp act-table load to
            # happen immediately instead of after the const DMA lands
            # (the auto-inserted LoadActFuncSet costs 1283ns on ACT).
            warm = cpool.tile([128, 1], f32, tag="warm")
            nc.gpsimd.memset(warm[:], 0.0)
            warm_o = cpool.tile([128, 1], f16, tag="warmo")
            nc.scalar.activation(warm_o[:], warm[:], Act.Exp, bias=0.0, scale=1.0)

            ct = cpool.tile([128, CONST_COLS + COLS], f16, tag="cx")
            nc.sync.dma_start(out=ct[:], in_=cx[:])
            wx_t = ct[:, 0:128]
            bias_t = ct[:, 128:130].bitcast(f32)

            # Prefetch every input chunk up front (one tile each, no reuse).
            # The first chunk rides the Pool/SWDGE path: its descriptor
            # generation runs on the idle Pool engine in parallel with the
            # const DMA's HWDGE slot, landing pass-1 data ~140ns earlier.
            xts = {0: (ct, None)}
            for ci, (p0, p1) in enumerate(IN_CHUNKS):
                xt = xpool.tile([128, (p1 - p0) * COLS], f16, tag=f"x{ci}")
                eng = nc.gpsimd if ci == 0 else nc.sync
                eng.dma_start(out=xt[:], in_=xin[:, p0 * COLS:p1 * COLS])
                for s in range(p0, p1):
                    xts[s] = (xt, s - p0)

            def emit_tree(et, out_slice, ma, mb, tag=""):
                # Binary-tree K-reduction over groups [ma, mb) of one pass.
                nm = mb - ma
                ev = et[:].rearrange("q (m k h) -> q m k h", m=NGRP, k=K, h=H)
                src = ev[:, ma:mb, :, :]
                for li, kk in enumerate((16, 8, 4, 2)):
                    tl = tpool.tile([128, nm * kk * H], f16, tag=f"t{li}{tag}")
                    tv = tl[:].rearrange("q (m k h) -> q m k h", m=nm, k=kk, h=H)
                    nc.vector.tensor_tensor(
                        tv, src[:, :, 0:kk, :], src[:, :, kk:2 * kk, :], Alu.add)
                    src = tv
                ov = out_slice.rearrange("q (m h) -> q m h", m=nm, h=H)
                nc.vector.tensor_tensor(ov, src[:, :, 0, :], src[:, :, 1, :], Alu.add)

            # Job list: (pass, group range), each job with its own PSUM
            # tile (within-job act segments pay an ACT pipeline-refill gap;
            # separate tiles do not). Head fill: pass 0 opens with a single
            # group (first exp right after one matmul + the const DMA),
            # pass 1 runs as two halves sized to its chunk-arrival and
            # matmul-chain readiness, then pass 0's remainder. Tail drain:
            # pass 15 leaves a single-group exp last so the post-ACT work
            # (small tree + 64-col DMA) is minimal. Segment count is
            # deliberately minimal: each extra segment costs 185ns of act
            # init and tightens the ~1.2us PSUM release-to-next-act chain.
            jobs = [(0, 0, 1), (1, 0, 2), (1, 2, 4), (0, 1, 4)] + [
                (s, 0, NGRP) for s in range(2, NPASS - 1)] + [
                (NPASS - 1, 0, 3), (NPASS - 1, 3, 4)]
            act_splits = {}
            tree_splits = {}

            out_iter = iter(OUT_CHUNKS)
            o0, o1 = next(out_iter)
            out_t = opool.tile([128, (o1 - o0) * 64], f16, tag="out")
            for s, j0, j1 in jobs:
                xt, si = xts[s]
                if si is None:
                    xs = xt[:, CONST_COLS:CONST_COLS + COLS]
                else:
                    xs = xt[:, si * COLS:(si + 1) * COLS]
                pt = ppool.tile([128, NGRP * COLS], f32, tag="acc")
                et = epool.tile([128, NGRP * COLS], f16, tag="e")
                # All matmuls precede the act segments in program order:
                # WAR deps are tile-granular, so a matmul emitted after an
                # activation read of the same PSUM tile would falsely wait
                # for it. RAW deps are subtile-granular, so each act segment
                # still starts as soon as its own groups' matmuls land.
                for m in range(j0, j1):
                    sl = slice(m * 32, (m + 1) * 32)
                    nc.tensor.matmul(
                        pt[:, (m - j0) * COLS:(m - j0 + 1) * COLS],
                        wx_t[sl, :], xs[sl, :],
                        start=True, stop=True, tile_position=(m * 32, 0),
                    )
                ma = 0
                for mb in act_splits.get(s, (j1 - j0,)):
                    nc.scalar.activation(
                        et[:, ma * COLS:mb * COLS], pt[:, ma * COLS:mb * COLS],
                        Act.Exp, bias=bias_t, scale=1.0)
                    ma = mb

                ma = 0
                for gi, mb in enumerate(tree_splits.get(s, (j1 - j0,))):
                    emit_tree(
                        et,
                        out_t[:, (s - o0) * 64 + (j0 + ma) * H:
                              (s - o0) * 64 + (j0 + mb) * H],
                        ma, mb, chr(ord("a") + gi) if gi else "")
                    ma = mb

                if s == o1 - 1 and j1 == NGRP:
                    nc.sync.dma_start(out=out[:, o0 * 64:o1 * 64], in_=out_t[:])
                    if s < NPASS - 1:
                        o0, o1 = next(out_iter)
                        out_t = opool.tile([128, (o1 - o0) * 64], f16, tag="out")

    nc.finalize()
    return nc


def _host_pack(x):
    """x: (B, N, K, C) fp32 -> per-batch rhs layout (B, 128, NPASS*COLS) fp16.

    D[b, m*32 + j*4 + cs, s*COLS + k*H + h] = v[b, n, k, cs]
    with n = s*512 + m*128 + h*8 + j,
    v[..., :3] = x, v[..., 3] = ||x||^2.
    """
    xr = x.reshape(B, NPASS, NGRP, H, NSLOT, K, C)
    xp = np.empty((B, NPASS, NGRP, H, NSLOT, K, SLOTS), dtype=np.float16)
    xp[..., :C] = xr.astype(np.float16)
    xp[..., C] = (xr * xr).sum(-1).astype(np.float16)
    # (b, s, m, h, j, k, cs) -> (b, m, j, cs, s, k, h)
    d = xp.transpose(0, 2, 4, 6, 1, 5, 3)
    return np.ascontiguousarray(d.reshape(B, 128, NPASS * COLS))


def _host_weights(kernel_points, sigma):
    kp = np.asarray(kernel_points, dtype=np.float64)
    s2 = float(sigma) ** 2
    a = -1.0 / (2.0 * s2)
    bcoef = kp / s2                                   # (P, C)
    ksq = (kp ** 2).sum(-1)                           # (P,)

    wx = np.zeros((128, 128), dtype=np.float16)
    for m in range(NGRP):
        for j in range(NSLOT):
            for cs in range(C):
                row = m * 32 + j * 4 + cs
                wx[row, j * 16:(j + 1) * 16] = bcoef[:, cs].astype(np.float16)
            wx[m * 32 + j * 4 + C, j * 16:(j + 1) * 16] = np.float16(a)
    bias = np.zeros((128, 1), dtype=np.float32)
    for j in range(NSLOT):
        bias[j * 16:(j + 1) * 16, 0] = (-ksq / (2.0 * s2) - np.log(K)).astype(np.float32)
    return wx, bias


def _host_cx(wx, bias, d):
    """Assemble the leading const DMA: [wx | bias f32-bitcast | pass-0 data].

    d: (B, 128, NPASS*COLS) packed input. Returns (B, 128, CONST_COLS+COLS).
    """
    cx = np.empty((B, 128, CONST_COLS + COLS), dtype=np.float16)
    cx[:, :, 0:128] = wx
    cx[:, :, 128:130] = bias.view(np.float16)
    cx[:, :, CONST_COLS:] = d[:, :, :COLS]
    return cx


def _host_unpack(outs):
    """outs: list of 8 per-core arrays (128, NPASS*64) f16 -> (B, N, P) f32."""
    res = np.empty((B, N, P), dtype=np.float32)
    for b, o in enumerate(outs):
        # o[j*16+p, s*64 + m*16 + h] = out[b, n, p], n = s*512 + m*128 + h*8 + j
        r = o.astype(np.float32).reshape(NSLOT, P, NPASS, NGRP, H)
        r = r.transpose(2, 3, 4, 0, 1)                    # (s, m, h, j, p)
        res[b] = r.reshape(N, P)
    return res


def kernel(neighborhoods, kernel_points, sigma):
    from concourse.bass_utils import run_bass_kernel_spmd

    x = np.asarray(neighborhoods, dtype=np.float32)
    d = _host_pack(x)
    wx, bias = _host_weights(kernel_points, sigma)
    cx = _host_cx(wx, bias, d)

    if "nc" not in _CACHE:
        _CACHE["nc"] = _build_nc()
    nc = _CACHE["nc"]

    core_ids = list(range(B))
    in_maps = [
        {"cx": cx[b], "xin": d[b]}
        for b in range(B)
    ]
    res = run_bass_kernel_spmd(nc, in_maps, core_ids)
    return _host_unpack([res.results[b]["out"] for b in range(B)])
